# revision 1
# baseline (speedup 1.0000x reference)
"""Trainium2 Bass kernel for nn_LinearTransformer (linear attention, 4 layers x 8 heads).

Math: each layer computes Z += sum_j (Z Qf_j Z^T)(mask . Z Pf_j^T)/(N-1), which
factorizes exactly (linear attention):
    Z_{l+1} = Z_l (I + A_l),   A_l = sum_j Qf_j G'_l Pf_j^T / (N-1)
    G'_l = Z_l^T Z_l - z_l z_l^T   (z_l = last token row)
Right-multiplicative layers collapse: Z_l = Z_0 C_l, and with
H_l = C_l^T G'_0 C_l (symmetric), D_l = C_l^T:
    U_l   = H_l @ PTs_l                     (PTs = scaled P_full^T blocks)
    A_l   = sum_j Qf_j U_{l,j}              (PSUM accumulation)
    IA    = I + A_l
    H_l+1 = IA^T (H_l IA)                   (two matmuls, H stays symmetric)
    D_l+1 = IA^T D_l
    Z_out = Z_0 C_4 = Z_0 D_4^T
The device streams Z only twice (Gram + final product); everything else is 64x64.

Sharding: data-parallel over batch B=16 across 8 cores (2 batches/core, no
collectives). Middle recurrence runs as two engine-parallel chains (batch 0
copies on DVE, batch 1 on ACT).
"""

import os
import numpy as np

B, N, D = 16, 2048, 64
NL, NH, DP = 4, 8, 63
NCORES = 8
BPC = B // NCORES  # 2 batches per core
NCHUNK = N // 128  # 16
NQ = 4  # DMA quarters
CPQ = NCHUNK // NQ  # chunks per quarter
SCALE = 1.0 / (N - 1)

_cache = {}


def _build():
    import concourse.bass as bass
    import concourse.mybir as mybir
    import concourse.tile as tile
    from concourse import bacc
    from concourse.masks import make_identity

    f32 = mybir.dt.float32

    nc = bacc.Bacc(
        "TRN2",
        target_bir_lowering=False,
        debug=False,
        enable_asserts=True,
        num_devices=NCORES,
    )

    Zd = nc.dram_tensor("Z", [BPC, N, D], f32, kind="ExternalInput")
    PTd = nc.dram_tensor("PT", [D, NL, 512], f32, kind="ExternalInput")
    QTd = nc.dram_tensor("QT", [D, NL, 512], f32, kind="ExternalInput")
    Od = nc.dram_tensor("O", [BPC, N, D], f32, kind="ExternalOutput")

    with tile.TileContext(nc) as tc:
        with (
            tc.tile_pool(name="const", bufs=1) as const,
            tc.tile_pool(name="zbuf", bufs=1) as zbuf,
            tc.tile_pool(name="mid", bufs=3) as mid,
            tc.tile_pool(name="pbig", bufs=2, space="PSUM") as pbig,
            tc.tile_pool(name="pacc", bufs=1, space="PSUM") as pacc,
            tc.tile_pool(name="pmix", bufs=4, space="PSUM") as pmix,
        ):
            ident = const.tile([128, 128], f32)
            make_identity(nc, ident)
            i64 = ident[0:64, 0:64]

            # last-token rows at partition 0 (rank-1 Gram correction), then Z
            # quarters on the SP queue; params on the gpsimd queue in parallel.
            zslab = const.tile([1, BPC, D], f32)
            ztq = []
            for q in range(NQ):
                zt = zbuf.tile([128, CPQ, BPC, D], f32, tag=f"zt{q}", name=f"zt{q}")
                ztq.append(zt)
                if q == 0:
                    # chunk 0 lands first so PE starts early
                    nc.sync.dma_start(
                        out=zt[:, 0, :, :], in_=Zd[:, 0:128, :].rearrange("b t d -> t b d")
                    )
                    for b in range(BPC):
                        nc.sync.dma_start(
                            out=zt[:, 1:, b, :],
                            in_=Zd[b, 128 : CPQ * 128, :].rearrange(
                                "(c t) d -> t c d", t=128
                            ),
                        )
                    nc.sync.dma_start(
                        out=zslab, in_=Zd[:, N - 1 : N, :].rearrange("b t d -> t b d")
                    )
                else:
                    qeng = {1: nc.sync, 2: nc.sync, 3: nc.sync}[q]
                    for b in range(BPC):
                        qeng.dma_start(
                            out=zt[:, :, b, :],
                            in_=Zd[b, q * CPQ * 128 : (q + 1) * CPQ * 128, :].rearrange(
                                "(c t) d -> t c d", t=128
                            ),
                        )
                if q == 0:
                    PTs = const.tile([D, NL, 512], f32)
                    nc.gpsimd.dma_start(out=PTs, in_=PTd[:, :, :])
                    QTs = const.tile([D, NL, 512], f32)
                    nc.gpsimd.dma_start(out=QTs, in_=QTd[:, :, :])

            negz = const.tile([1, BPC, D], f32)
            nc.vector.tensor_scalar_mul(negz, zslab, -1.0)

            # --- phase 1: Gram matrices (per batch, all base-0) + transposes ---
            Wstack = zbuf.tile([128, N], f32)  # [(b,d), token]
            pg = [pacc.tile([64, 64], f32, tag=f"pg{b}", name=f"pg{b}") for b in range(BPC)]
            for c in range(NCHUNK):
                zt = ztq[c // CPQ]
                cc = c % CPQ
                Zc = zt[:, cc, :, :].rearrange("p b d -> p (b d)")
                if c % 2 == 0:
                    pw = pbig.tile([128, 2, 128], f32, tag="big")
                    nc.tensor.transpose(pw[:, 0, :], Zc, ident)
                else:
                    nc.tensor.transpose(pw[:, 1, :], Zc, ident)
                for b in range(BPC):
                    nc.tensor.matmul(
                        pg[b],
                        lhsT=zt[:, cc, b, :],
                        rhs=zt[:, cc, b, :],
                        start=(c == 0),
                        stop=False,
                    )
                if c % 2 == 1:
                    eng = nc.vector if (c // 2) % 2 == 0 else nc.scalar
                    (eng.tensor_copy if eng is nc.vector else eng.copy)(
                        Wstack[:, (c - 1) * 128 : (c + 1) * 128],
                        pw.rearrange("p k a -> p (k a)"),
                    )
            # G -= z z^T
            Hs = [None, None]
            for b in range(BPC):
                nc.tensor.matmul(
                    pg[b],
                    lhsT=negz[0:1, b, :],
                    rhs=zslab[0:1, b, :],
                    start=False,
                    stop=True,
                )
            g0 = mid.tile([64, D], f32, tag="h0")
            nc.vector.tensor_copy(g0, pg[0])
            g1 = mid.tile([64, D], f32, tag="h1")
            nc.scalar.copy(g1, pg[1])
            Hs = [g0, g1]

            # --- middle recurrence: two engine-parallel chains ---
            cp = [
                lambda o, i: nc.vector.tensor_copy(o, i),
                lambda o, i: nc.scalar.copy(o, i),
            ]
            Ds = [None, None]
            for l in range(NL):
                pU, Us, pA, IAs, pR, Rs, pD, pH = (
                    [None] * 2, [None] * 2, [None] * 2, [None] * 2,
                    [None] * 2, [None] * 2, [None] * 2, [None] * 2,
                )
                for b in range(BPC):
                    pU[b] = pmix.tile([64, 512], f32, tag="mid", name=f"pU{b}_{l}")
                    nc.tensor.matmul(
                        pU[b], lhsT=Hs[b], rhs=PTs[:, l, :], start=True, stop=True
                    )
                for b in range(BPC):
                    Us[b] = mid.tile([64, 512], f32, tag=f"us{b}", name=f"us{b}_{l}")
                    cp[b](Us[b], pU[b])
                for b in range(BPC):
                    pA[b] = pmix.tile([64, 64], f32, tag="mid", name=f"pA{b}_{l}")
                    for j in range(NH):
                        nc.tensor.matmul(
                            pA[b],
                            lhsT=QTs[:, l, j * 64 : (j + 1) * 64],
                            rhs=Us[b][:, j * 64 : (j + 1) * 64],
                            start=(j == 0),
                            stop=(j == NH - 1),
                        )
                for b in range(BPC):
                    # IA = I + A, fused into the PSUM drain (ACT cannot do
                    # tensor+tensor, so both adds ride DVE)
                    IAs[b] = mid.tile([64, D], f32, tag=f"ia{b}", name=f"ia{b}_{l}")
                    nc.vector.tensor_add(IAs[b], i64, pA[b])
                # PE: R (skip last layer), D updates
                if l < NL - 1:
                    for b in range(BPC):
                        pR[b] = pmix.tile([64, D], f32, tag="mid", name=f"pR{b}_{l}")
                        nc.tensor.matmul(
                            pR[b], lhsT=Hs[b], rhs=IAs[b], start=True, stop=True
                        )
                for b in range(BPC):
                    pD[b] = pmix.tile([64, D], f32, tag="mid", name=f"pD{b}_{l}")
                    nc.tensor.matmul(
                        pD[b],
                        lhsT=IAs[b],
                        rhs=(Ds[b] if l > 0 else i64),
                        start=True,
                        stop=True,
                    )
                if l < NL - 1:
                    for b in range(BPC):
                        Rs[b] = mid.tile([64, D], f32, tag=f"rs{b}", name=f"rs{b}_{l}")
                        cp[b](Rs[b], pR[b])
                for b in range(BPC):
                    Ds[b] = mid.tile([64, D], f32, tag=f"ds{b}", name=f"ds{b}_{l}")
                    cp[b](Ds[b], pD[b])
                if l < NL - 1:
                    for b in range(BPC):
                        pH[b] = pmix.tile([64, D], f32, tag="mid", name=f"pH{b}_{l}")
                        nc.tensor.matmul(
                            pH[b], lhsT=IAs[b], rhs=Rs[b], start=True, stop=True
                        )
                    for b in range(BPC):
                        Hs[b] = mid.tile([64, D], f32, tag=f"h{b}", name=f"hn{b}_{l}")
                        cp[b](Hs[b], pH[b])

            # --- C4 = D4^T per batch, assembled block-diagonally ---
            pce = pmix.tile([128, D], f32, tag="mid")
            nc.tensor.matmul(pce[0:64, :], lhsT=Ds[0], rhs=i64, start=True, stop=True)
            nc.tensor.matmul(pce[64:128, :], lhsT=Ds[1], rhs=i64, start=True, stop=True)
            C4blk = mid.tile([128, BPC * D], f32, tag="c4")
            nc.gpsimd.memset(C4blk, 0.0)
            nc.vector.tensor_copy(C4blk[0:64, 0:D], pce[0:64, :])
            nc.scalar.copy(C4blk[64:128, D : 2 * D], pce[64:128, :])

            # --- Z_out = Z C4, streamed back by quarters ---
            for q in range(NQ):
                zo = zbuf.tile([128, CPQ, BPC, D], f32, tag=f"zo{q}", name=f"zo{q}")
                for c2 in range(CPQ // 2):
                    po = pbig.tile([128, 2, BPC * D], f32, tag="big")
                    for k in range(2):
                        c = q * CPQ + 2 * c2 + k
                        nc.tensor.matmul(
                            po[:, k, :],
                            lhsT=Wstack[:, c * 128 : (c + 1) * 128],
                            rhs=C4blk,
                            start=True,
                            stop=True,
                        )
                    eng_i = (q * (CPQ // 2) + c2) % 2
                    if eng_i == 0:
                        nc.vector.tensor_copy(
                            zo[:, 2 * c2 : 2 * c2 + 2, :, :],
                            po.rearrange("t k (b d) -> t k b d", b=BPC),
                        )
                    else:
                        nc.scalar.copy(
                            zo[:, 2 * c2 : 2 * c2 + 2, :, :],
                            po.rearrange("t k (b d) -> t k b d", b=BPC),
                        )
                for b in range(BPC):
                    nc.sync.dma_start(
                        out=Od[b, q * CPQ * 128 : (q + 1) * CPQ * 128, :].rearrange(
                            "(c t) d -> t c d", t=128
                        ),
                        in_=zo[:, :, b, :],
                    )

    nc.compile()
    return nc


def _get_nc():
    if "nc" not in _cache:
        _cache["nc"] = _build()
    return _cache["nc"]


def _host_params(allparam):
    ap = np.asarray(allparam, dtype=np.float32)
    Pf = np.zeros((NL, NH, D, D), np.float32)
    Qf = np.zeros((NL, NH, D, D), np.float32)
    Pf[:, :, :DP, :DP] = ap[:, :, 0]
    Pf[:, :, DP, DP] = 1.0
    Qf[:, :, :DP, :DP] = ap[:, :, 1]
    # PT[d, l, j*64+e] = Pf[l,j,e,d] * SCALE  (P_full^T blocks side by side)
    PT = np.ascontiguousarray(
        (Pf.transpose(3, 0, 1, 2) * SCALE).reshape(D, NL, NH * D)
    )
    QT = np.ascontiguousarray(Qf.transpose(3, 0, 1, 2).reshape(D, NL, NH * D))
    return PT, QT


def kernel(Z, allparam):
    from concourse.bass_utils import run_bass_kernel_spmd

    Z = np.ascontiguousarray(np.asarray(Z, dtype=np.float32))
    PT, QT = _host_params(allparam)
    nc = _get_nc()

    in_maps = []
    for core in range(NCORES):
        zshard = np.ascontiguousarray(Z[core * BPC : (core + 1) * BPC])
        in_maps.append({"Z": zshard, "PT": PT, "QT": QT})

    res = run_bass_kernel_spmd(
        nc,
        in_maps,
        core_ids=list(range(NCORES)),
        trace=bool(int(os.environ.get("KERNEL_TRACE", "0") or "0")),
    )
    _cache["last_results"] = res

    out = np.empty((B, N, D), np.float32)
    for core in range(NCORES):
        out[core * BPC : (core + 1) * BPC] = res.results[core]["O"]
    return out



# revision 16
# speedup vs baseline: 1.5657x; 1.5657x over previous
"""Trainium2 Bass kernel for nn_LinearTransformer (linear attention, 4 layers x 8 heads).

Math: each layer computes Z += sum_j (Z Qf_j Z^T)(mask . Z Pf_j^T)/(N-1), which
factorizes exactly (linear attention):
    Z_{l+1} = Z_l (I + A_l),   A_l = sum_j Qf_j G'_l Pf_j^T / (N-1)
    G'_l = Z_l^T Z_l - z_l z_l^T   (z_l = last token row)
Right-multiplicative layers collapse: Z_l = Z_0 C_l, and with
H_l = C_l^T G'_0 C_l (symmetric), D_l = C_l^T:
    U_l = H_l @ PTs_l, A_l = sum_j Qf_j U_{l,j}, IA = I + A_l
    H_{l+1} = IA^T (H_l IA), D_{l+1} = IA^T D_l, C_4 = D_3^T IA_4
    Z_out = Z_0 C_4
All matmuls run in bf16 (Z is converted to bf16 on the HOST, so the device
streams 2 bytes/elem in and computes immediately); PSUM accumulates fp32 and
the final product drains to fp32 for the output. Tokens are packed 4-per-row
so every DMA descriptor is >= 512B (full DMA bus efficiency). The two batches
per core are stacked side-by-side in [64, 128] tiles so the whole layer
recurrence runs as one chain of wide ops; batch 0 drains ride DVE, batch 1
rides ACT. I + A is formed by seeding the PSUM accumulation with an identity
matmul (I = I64^T @ I64) before the 8 per-head matmuls, so the drain is a
plain copy. Transposes of Z (needed as lhsT for the final product) fill PE
idle slots inside the recurrence.

Sharding: data-parallel over batch B=16 across 8 cores (2 batches/core, no
collectives).
"""

import os
import numpy as np

B, N, D = 16, 2048, 64
NL, NH, DP = 4, 8, 63
NCORES = 8
BPC = B // NCORES  # 2 batches per core
NC4 = 4  # chunks of 128 packed rows (4 tokens each)
SCALE = 1.0 / (N - 1)

_cache = {}


def _build(stage=None):
    if stage is None:
        stage = int(os.environ.get("KERNEL_STAGE", "4") or "4")
    import concourse.bass as bass
    import concourse.mybir as mybir
    import concourse.tile as tile
    from concourse import bacc
    from concourse.masks import make_identity

    f32 = mybir.dt.float32
    bf16 = mybir.dt.bfloat16

    nc = bacc.Bacc(
        "TRN2",
        target_bir_lowering=False,
        debug=False,
        enable_asserts=True,
        num_devices=NCORES,
    )

    # Z packed 4 tokens per 512B row: [b, 512, 256] bf16
    Zd = nc.dram_tensor("Z", [BPC, 512, 256], bf16, kind="ExternalInput")
    # PT[d, l, j*64+e] = Pf[l,j,e,d]*SCALE
    PTd = nc.dram_tensor("PT", [D, NL, 512], bf16, kind="ExternalInput")
    # QT[d, l, j*64+a] = Qf[l,j,a,d]
    QTd = nc.dram_tensor("QT", [D, NL, 512], bf16, kind="ExternalInput")
    # ZL[0, b, 0:64] = z_last of batch b, ZL[0, b, 64:128] = -z_last
    ZLd = nc.dram_tensor("ZL", [1, BPC, 128], bf16, kind="ExternalInput")
    Od = nc.dram_tensor("O", [BPC, 512, 256], f32, kind="ExternalOutput")

    with tile.TileContext(nc) as tc:
        with (
            tc.tile_pool(name="const", bufs=1) as const,
            tc.tile_pool(name="zbuf", bufs=1) as zbuf,
            tc.tile_pool(name="mid", bufs=2) as mid,
            tc.tile_pool(name="pu", bufs=1, space="PSUM") as pu,
            tc.tile_pool(name="psm", bufs=2, space="PSUM") as psm,
            tc.tile_pool(name="pt", bufs=1, space="PSUM") as pt,
            tc.tile_pool(name="po", bufs=2, space="PSUM") as po,
        ):
            ident = const.tile([128, 128], bf16)
            make_identity(nc, ident)
            i64 = ident[0:64, 0:64]
            # [I64 | I64]: single-matmul identity seed for the I + A trick
            ident2 = const.tile([64, 2, 64], bf16)
            nc.gpsimd.memset(ident2, 0.0)
            nc.gpsimd.affine_select(
                out=ident2,
                in_=ident2,
                compare_op=mybir.AluOpType.not_equal,
                fill=1.0,
                base=0,
                pattern=[[0, 2], [-1, 64]],
                channel_multiplier=1,
            )

            # --- loads ---
            Zt = []
            for b in range(BPC):
                zt = zbuf.tile([128, NC4, 256], bf16, tag=f"zt{b}", name=f"zt{b}")
                nc.sync.dma_start(
                    out=zt, in_=Zd[b].rearrange("(c p) f -> p c f", p=128)
                )
                Zt.append(zt)
            ZLs = const.tile([1, BPC, 128], bf16)
            nc.gpsimd.dma_start(out=ZLs, in_=ZLd[:, :, :])
            PTs = const.tile([D, NL, 512], bf16)
            nc.gpsimd.dma_start(out=PTs, in_=PTd[:, :, :])
            QTs = const.tile([D, NL, 512], bf16)
            nc.gpsimd.dma_start(out=QTs, in_=QTd[:, :, :])

            # NOTE (hardware, not modeled by CoreSim): all matmul groups that
            # target the same PSUM bank must use the same operand base
            # partition. Everything below keeps PSUM-bank populations
            # single-base (base 0).

            # --- Gram: G_b = Z_b^T Z_b - z_last z_last^T, to Glhs [64,(b,d)] ---
            # ZL carries [+z_last | -z_last] per batch (host-packed), so the
            # rank-1 correction rides the same PSUM accumulation group and the
            # drain is a plain copy.
            pg = [
                psm.tile([64, 64], f32, tag="pdrh", bufs=2, name=f"pg{b}")
                for b in range(BPC)
            ]
            for b in range(BPC):
                k = 0
                for c in range(NC4):
                    for q in range(4):
                        nc.tensor.matmul(
                            pg[b],
                            lhsT=Zt[b][:, c, q * 64 : (q + 1) * 64],
                            rhs=Zt[b][:, c, q * 64 : (q + 1) * 64],
                            start=(k == 0),
                            stop=False,
                        )
                        k += 1
                nc.tensor.matmul(
                    pg[b],
                    lhsT=ZLs[0:1, b, 64:128],
                    rhs=ZLs[0:1, b, 0:64],
                    start=False,
                    stop=True,
                )
            Glhs = mid.tile([64, BPC * 64], bf16, tag="h", name="glhs")
            for b in range(BPC):
                nc.vector.tensor_copy(Glhs[:, b * 64 : (b + 1) * 64], pg[b])

            # --- recurrence over layers (tiny 64x64 algebra) ---
            Hs = Glhs
            Ds = None
            C4 = None
            tp_idx = 0  # transpose work interleaved into PE idle slots
            # Wt[b][:, c, k, :] = z^T for tokens 4r+k of chunk c (all base 0)
            Wt = [
                zbuf.tile([64, NC4, 4, 128], bf16, tag=f"wt{b}", name=f"wt{b}")
                for b in range(BPC)
            ]

            def emit_transposes(n):
                nonlocal tp_idx
                for _ in range(n):
                    if tp_idx >= BPC * NC4:
                        return
                    b, c = divmod(tp_idx, NC4)
                    ptt = pt.tile([64, 4, 128], bf16, tag="pt", name=f"ptt{tp_idx}")
                    for q in range(4):
                        nc.tensor.transpose(
                            ptt[:, q, :], Zt[b][:, c, q * 64 : (q + 1) * 64], ident
                        )
                    nc.scalar.copy(Wt[b][:, c, :, :], ptt)
                    tp_idx += 1

            nlr = min(NL, stage) if stage >= 0 else 0
            for l in range(nlr):
                last = l == nlr - 1
                # U_b = H_b @ PTs, one matmul per batch (base-0 operands)
                pU = [
                    pu.tile([64, 512], f32, tag=f"pu{b}", name=f"pU{b}_{l}")
                    for b in range(BPC)
                ]
                for b in range(BPC):
                    nc.tensor.matmul(
                        pU[b], lhsT=Hs[:, b * 64 : (b + 1) * 64],
                        rhs=PTs[:, l, :], start=True, stop=True,
                    )
                # pA seeded with identity so pA = I + sum_j Qf_j U_j after heads
                pA = psm.tile([64, BPC * 64], f32, tag="pa", bufs=1, name=f"pA_{l}")
                nc.tensor.matmul(
                    pA, lhsT=i64, rhs=ident2.rearrange("p a b -> p (a b)"),
                    start=True, stop=False,
                )
                emit_transposes(1)  # fills PE idle while U drains
                Ubf = [
                    mid.tile([64, 512], bf16, tag=f"ubf{b}", name=f"ubf{b}_{l}")
                    for b in range(BPC)
                ]
                nc.vector.tensor_copy(Ubf[0], pU[0])
                nc.scalar.copy(Ubf[1], pU[1])
                # A_b += sum_j QT_j @ U_bj
                for b in range(BPC):
                    for j in range(NH):
                        nc.tensor.matmul(
                            pA[:, b * 64 : (b + 1) * 64],
                            lhsT=QTs[:, l, j * 64 : (j + 1) * 64],
                            rhs=Ubf[b][:, j * 64 : (j + 1) * 64],
                            start=False,
                            stop=(b == BPC - 1 and j == NH - 1),
                        )
                IA = mid.tile([64, BPC * 64], bf16, tag="ia", name=f"ia_{l}")
                nc.vector.tensor_copy(IA, pA)
                emit_transposes(1)  # fills PE idle while IA drains

                if not last:
                    # R = H IA ; H' = IA^T R  (critical chain, drains on DVE)
                    pR = psm.tile([64, BPC * 64], f32, tag="pdrh", bufs=2,
                                  name=f"pR_{l}")
                    for b in range(BPC):
                        bs = slice(b * 64, (b + 1) * 64)
                        nc.tensor.matmul(pR[:, bs], lhsT=Hs[:, bs], rhs=IA[:, bs],
                                         start=True, stop=True)
                    Rbf = mid.tile([64, BPC * 64], bf16, tag="rbf", name=f"rbf_{l}")
                    nc.vector.tensor_copy(Rbf, pR)
                    pH = psm.tile([64, BPC * 64], f32, tag="pdrh", bufs=2,
                                  name=f"pH_{l}")
                    for b in range(BPC):
                        bs = slice(b * 64, (b + 1) * 64)
                        nc.tensor.matmul(pH[:, bs], lhsT=IA[:, bs], rhs=Rbf[:, bs],
                                         start=True, stop=True)
                    Hn = mid.tile([64, BPC * 64], bf16, tag="h", name=f"h_{l}")
                    nc.vector.tensor_copy(Hn, pH)
                # D' = IA^T D (D_0 = IA_0^T); last layer C4 = D_3^T IA_4
                pD = psm.tile([64, BPC * 64], f32, tag="pdrh", bufs=2,
                              name=f"pD_{l}")
                for b in range(BPC):
                    bs = slice(b * 64, (b + 1) * 64)
                    if last:
                        nc.tensor.matmul(
                            pD[:, bs], lhsT=(Ds[:, bs] if l > 0 else i64),
                            rhs=IA[:, bs], start=True, stop=True,
                        )
                    elif l == 0:
                        nc.tensor.matmul(pD[:, bs], lhsT=IA[:, bs], rhs=i64,
                                         start=True, stop=True)
                    else:
                        nc.tensor.matmul(pD[:, bs], lhsT=IA[:, bs], rhs=Ds[:, bs],
                                         start=True, stop=True)
                if last:
                    C4 = mid.tile([64, BPC * 64], bf16, tag="c4", name="c4")
                    nc.vector.tensor_copy(C4, pD)
                else:
                    Dn = mid.tile([64, BPC * 64], bf16, tag="dbf", name=f"d_{l}")
                    nc.scalar.copy(Dn, pD)
                    Ds = Dn
                    Hs = Hn
            emit_transposes(BPC * NC4)  # any remainder
            if C4 is None:
                C4 = ident[0:64, 0:128]  # bypass recurrence (debug stages)

            # --- Z_out = Z @ C4, streamed back per half-batch ---
            zdr = [
                lambda o, i: nc.vector.tensor_copy(o, i),
                lambda o, i: nc.scalar.copy(o, i),
            ]
            for b in range(BPC):
                zo = zbuf.tile([128, NC4, 256], f32, tag=f"zo{b}", name=f"zo{b}")
                bs = slice(b * 64, (b + 1) * 64)
                for c in range(NC4):
                    pOt = po.tile([128, 4, 64], f32, tag="po", name=f"po{b}_{c}")
                    for k in range(4):
                        nc.tensor.matmul(
                            pOt[:, k, :],
                            lhsT=Wt[b][:, c, k, :],
                            rhs=C4[:, bs],
                            start=True,
                            stop=True,
                        )
                    zdr[(b * NC4 + c) % 2](
                        zo[:, c, :], pOt.rearrange("p k d -> p (k d)")
                    )
                    if c % 2 == 1:
                        eng = nc.sync if (b * 2 + c // 2) % 2 == 0 else nc.gpsimd
                        eng.dma_start(
                            out=Od[b, (c - 1) * 128 : (c + 1) * 128, :].rearrange(
                                "(c p) f -> p c f", p=128
                            ),
                            in_=zo[:, c - 1 : c + 1, :],
                        )

    nc.compile()
    return nc


def _get_nc():
    if "nc" not in _cache:
        _cache["nc"] = _build()
    return _cache["nc"]


def _host_params(allparam):
    import ml_dtypes

    ap = np.asarray(allparam, dtype=np.float32)
    Pf = np.zeros((NL, NH, D, D), np.float32)
    Qf = np.zeros((NL, NH, D, D), np.float32)
    Pf[:, :, :DP, :DP] = ap[:, :, 0]
    Pf[:, :, DP, DP] = 1.0
    Qf[:, :, :DP, :DP] = ap[:, :, 1]
    PT = (Pf.transpose(3, 0, 1, 2) * SCALE).reshape(D, NL, NH * D)
    QT = Qf.transpose(3, 0, 1, 2).reshape(D, NL, NH * D)
    PT = np.ascontiguousarray(PT.astype(ml_dtypes.bfloat16))
    QT = np.ascontiguousarray(QT.astype(ml_dtypes.bfloat16))
    return PT, QT


def kernel(Z, allparam):
    import ml_dtypes
    from concourse.bass_utils import run_bass_kernel_spmd

    Zbf = np.asarray(Z, dtype=np.float32).astype(ml_dtypes.bfloat16)
    PT, QT = _host_params(allparam)
    nc = _get_nc()

    in_maps = []
    for core in range(NCORES):
        zs = Zbf[core * BPC : (core + 1) * BPC]
        zshard = np.ascontiguousarray(zs.reshape(BPC, 512, 256))
        zl = np.empty((1, BPC, 128), Zbf.dtype)
        zl[0, :, 0:64] = zs[:, N - 1, :]
        zl[0, :, 64:128] = -zs[:, N - 1, :]
        in_maps.append({"Z": zshard, "PT": PT, "QT": QT, "ZL": zl})

    res = run_bass_kernel_spmd(
        nc,
        in_maps,
        core_ids=list(range(NCORES)),
        trace=bool(int(os.environ.get("KERNEL_TRACE", "0") or "0")),
    )
    _cache["last_results"] = res

    out = np.empty((B, N, D), np.float32)
    for core in range(NCORES):
        out[core * BPC : (core + 1) * BPC] = res.results[core]["O"].reshape(
            BPC, N, D
        )
    return out
